# revision 18
# baseline (speedup 1.0000x reference)
"""Trainium2 Bass kernel for masked-attention transformer block.

Computes, per batch item b (B=256, S=512, D_IN=256, D_ATT=512):
    Q = x@Wq + bq + pe;  K = x@Wk + bk + pe;  V = x@Wv + bv + pe
    scores = Q K^T / sqrt(D);  scores[:, k >= mask_start[b]] = -inf
    attn = softmax(scores);  o = attn@V + V;  y = LN(o) * gamma + beta
    out = y@Wf + bf + y

Sharding: data-parallel over batch, 32 items per core across 8 cores.

Strategy (per item):
  - matmuls in bf16 (fp32 is 4 cycles/row on the PE, bf16 is 1);
    accumulation stays fp32 in PSUM; softmax normalization, residuals
    and layernorm statistics stay fp32.
  - x^T via PE transposes (DMA transpose needs 16-bit; done post-cast)
  - Q^T,K^T computed as [d, s] (lhsT=W, rhs=x^T); V natural [s, d]
    (lhsT=x^T, rhs=Wv)
  - scores^T [k, q] = (K^T-slice as lhsT) @ Q^T; key-mask + 1/sqrt(D)
    fused into the Exp activation (per-partition bias/scale) -> E^T
  - numerator = (E^T-slice as lhsT) @ V -> [q, d]; denominator from an
    extra rhs=ones column matmul -> [q, 1] (per-partition)
  - o = num * (1/den) + V in one fused scalar_tensor_tensor DVE op
  - LN stats via bn_stats/bn_aggr; rsqrt on idle GpSimd via pow;
    y_pre = (o - mu) * rs on GpSimd, output directly in bf16
  - gamma/beta folded host-side: Wg2 = diag(gamma') @ Wf + diag(gamma),
    c = beta@Wf + bf + beta, so out = y_pre@Wg2 + c (one matmul + add)
  - y_pre transposed on PE (bf16) for the final matmul
"""

import numpy as np

import concourse.bass as bass
import concourse.tile as tile
from concourse import bacc, mybir
from concourse.bass_utils import run_bass_kernel_spmd

N_CORES = 8
B, S, D_IN, D_ATT = 256, 512, 256, 512
BPC = B // N_CORES
EPS = 1e-5
SCALE = float(1.0 / np.sqrt(D_ATT))
NEG = -30000.0
FP32 = mybir.dt.float32
BF16 = mybir.dt.bfloat16
P = 128
KI = D_IN // P   # 2  k-tiles over input dim
KS = S // P      # 4  tiles over seq
KD = D_ATT // P  # 4  tiles over attention dim

AF = mybir.ActivationFunctionType
OP = mybir.AluOpType

# set by test harness to capture profiling info
TRACE = False
LAST_RESULTS = None


def build_program(n_items):
    nc = bacc.Bacc(None, target_bir_lowering=False, debug=False)

    x_d = nc.dram_tensor("x", [n_items, S, D_IN], FP32, kind="ExternalInput")
    m_d = nc.dram_tensor("mstart", [1, n_items], FP32, kind="ExternalInput")
    wq_d = nc.dram_tensor("wq", [D_IN, D_ATT], BF16, kind="ExternalInput")
    wk_d = nc.dram_tensor("wk", [D_IN, D_ATT], BF16, kind="ExternalInput")
    wv_d = nc.dram_tensor("wv", [D_IN, D_ATT], BF16, kind="ExternalInput")
    wg2_d = nc.dram_tensor("wg2", [D_ATT, D_ATT], BF16, kind="ExternalInput")
    pbq_d = nc.dram_tensor("petbq", [D_ATT, S], FP32, kind="ExternalInput")
    pbk_d = nc.dram_tensor("petbk", [D_ATT, S], FP32, kind="ExternalInput")
    pbv_d = nc.dram_tensor("pebv", [S, D_ATT], FP32, kind="ExternalInput")
    cf_d = nc.dram_tensor("cfull", [P, D_ATT], FP32, kind="ExternalInput")
    io_d = nc.dram_tensor("iota4", [P, KS], FP32, kind="ExternalInput")
    id_d = nc.dram_tensor("ident", [P, P], BF16, kind="ExternalInput")
    id32_d = nc.dram_tensor("ident32", [P, P], FP32, kind="ExternalInput")
    out_d = nc.dram_tensor("out", [n_items, S, D_ATT], FP32, kind="ExternalOutput")

    with tile.TileContext(nc) as tc:
        with (
            tc.tile_pool(name="const", bufs=1) as cpool,
            tc.tile_pool(name="work", bufs=3) as wpool,
            tc.tile_pool(name="outp", bufs=4) as opool,
            tc.tile_pool(name="small", bufs=3) as spool,
            tc.tile_pool(name="psA", bufs=6, space="PSUM") as psA,
            tc.tile_pool(name="psB", bufs=2, space="PSUM") as psB,
        ):
            # ---------------- constants (loaded once) ----------------
            wq = cpool.tile([P, KI, D_ATT], BF16, name="wq_sb")
            nc.sync.dma_start(out=wq, in_=wq_d[:].rearrange("(k p) d -> p k d", p=P))
            wk = cpool.tile([P, KI, D_ATT], BF16, name="wk_sb")
            nc.sync.dma_start(out=wk, in_=wk_d[:].rearrange("(k p) d -> p k d", p=P))
            wv = cpool.tile([P, KI, D_ATT], BF16, name="wv_sb")
            nc.sync.dma_start(out=wv, in_=wv_d[:].rearrange("(k p) d -> p k d", p=P))
            wg2 = cpool.tile([P, KD, D_ATT], BF16, name="wg2_sb")
            nc.sync.dma_start(out=wg2, in_=wg2_d[:].rearrange("(k p) d -> p k d", p=P))
            pbq = cpool.tile([P, KD, S], FP32, name="pbq_sb")
            nc.sync.dma_start(out=pbq, in_=pbq_d[:].rearrange("(m p) s -> p m s", p=P))
            pbk = cpool.tile([P, KD, S], FP32, name="pbk_sb")
            nc.sync.dma_start(out=pbk, in_=pbk_d[:].rearrange("(m p) s -> p m s", p=P))
            pbv = cpool.tile([P, KS, D_ATT], FP32, name="pbv_sb")
            nc.sync.dma_start(out=pbv, in_=pbv_d[:].rearrange("(m p) d -> p m d", p=P))
            cf = cpool.tile([P, D_ATT], FP32, name="cf_sb")
            nc.sync.dma_start(out=cf, in_=cf_d[:])
            iota = cpool.tile([P, KS], FP32, name="iota_sb")
            nc.sync.dma_start(out=iota, in_=io_d[:])
            ident = cpool.tile([P, P], BF16, name="ident_sb")
            nc.sync.dma_start(out=ident, in_=id_d[:])
            ident32 = cpool.tile([P, P], FP32, name="ident32_sb")
            nc.sync.dma_start(out=ident32, in_=id32_d[:])

            ones_col = cpool.tile([P, 1], BF16, name="ones_col")
            nc.vector.memset(ones_col, 1.0)
            eps_t = cpool.tile([P, 1], FP32, name="eps_t")
            nc.vector.memset(eps_t, EPS)
            zero_t = cpool.tile([P, 1], FP32, name="zero_t")
            nc.vector.memset(zero_t, 0.0)

            # broadcast mask starts to all 128 partitions on GpSimd
            m_row = cpool.tile([1, n_items], FP32, name="m_row")
            nc.sync.dma_start(out=m_row, in_=m_d[:])
            m_bc = cpool.tile([P, n_items], FP32, name="m_bc")
            nc.gpsimd.partition_broadcast(m_bc, m_row)

            # ---------------- per-item pipeline ----------------
            # Software-pipelined emission: each engine's (in-order) stream
            # alternates FRONT(b) and TAIL(b-1) so there is always work from
            # two items in flight and the PE never idles past the HAM window.

            def front(b):
                x_nat = wpool.tile([P, KS, D_IN], FP32, tag="xnat", name=f"xnat{b}")
                nc.sync.dma_start(
                    out=x_nat, in_=x_d[b].rearrange("(t p) i -> p t i", p=P)
                )
                # additive key-mask column: 0 where k < m_b else -30000
                maskb = spool.tile([P, KS], FP32, tag="maskb", name=f"maskb{b}")
                nc.vector.tensor_scalar(
                    maskb, iota, m_bc[:, b : b + 1], NEG, OP.is_ge, OP.mult
                )

                # x^T via fp32 PE transposes; bf16 cast fused into psum copy
                xT = wpool.tile([P, KI, S], BF16, tag="xT", name=f"xT{b}")
                for ii in range(KI):
                    tp = psA.tile([P, S], FP32, tag="ps", name=f"tpx{b}_{ii}")
                    for ss in range(KS):
                        nc.tensor.transpose(
                            out=tp[:, P * ss : P * (ss + 1)],
                            in_=x_nat[:, ss, P * ii : P * (ii + 1)],
                            identity=ident32,
                        )
                    nc.scalar.copy(out=xT[:, ii, :], in_=tp)

                # projections: Q^T, K^T as [d, s] bf16; V natural [s, d] fp32
                QT = wpool.tile([P, KD, S], BF16, tag="QT", name=f"QT{b}")
                KTt = wpool.tile([P, KD, S], BF16, tag="KTt", name=f"KTt{b}")
                for dst, w, pb in ((QT, wq, pbq), (KTt, wk, pbk)):
                    for m in range(KD):
                        ps = psA.tile([P, S], FP32, tag="ps")
                        for k in range(KI):
                            nc.tensor.matmul(
                                ps,
                                lhsT=w[:, k, P * m : P * (m + 1)],
                                rhs=xT[:, k, :],
                                start=(k == 0),
                                stop=(k == KI - 1),
                            )
                        nc.vector.tensor_add(dst[:, m, :], ps, pb[:, m, :])
                Vbf = wpool.tile([P, KS, D_ATT], BF16, tag="Vbf", name=f"Vbf{b}")
                for m in range(KS):
                    ps = psA.tile([P, D_ATT], FP32, tag="ps")
                    for k in range(KI):
                        nc.tensor.matmul(
                            ps,
                            lhsT=xT[:, k, P * m : P * (m + 1)],
                            rhs=wv[:, k, :],
                            start=(k == 0),
                            stop=(k == KI - 1),
                        )
                    nc.vector.tensor_add(Vbf[:, m, :], ps, pbv[:, m, :])

                # scores^T [k, q] + fused mask/scale/exp -> E^T (bf16)
                ET = wpool.tile([P, KS, S], BF16, tag="ET", name=f"ET{b}")
                for t in range(KS):
                    ps = psA.tile([P, S], FP32, tag="ps")
                    for m in range(KD):
                        nc.tensor.matmul(
                            ps,
                            lhsT=KTt[:, m, P * t : P * (t + 1)],
                            rhs=QT[:, m, :],
                            start=(m == 0),
                            stop=(m == KD - 1),
                        )
                    nc.scalar.activation(
                        out=ET[:, t, :],
                        in_=ps,
                        func=AF.Exp,
                        bias=maskb[:, t : t + 1],
                        scale=SCALE,
                    )
                return ET, Vbf

            def tail(b, ET, Vbf):
                # attention output; layernorm of o/den == layernorm of o
                # (row-scale invariance), so no reciprocal: o'' = den*V + num
                # and eps is scaled by den^2 inside the sqrt.
                t2 = wpool.tile([P, KS, D_ATT], BF16, tag="t2", name=f"t2{b}")
                o4 = wpool.tile([P, KS, D_ATT], FP32, tag="o4", name=f"o4{b}")
                den_sb = spool.tile([P, KS], FP32, tag="den_sb", name=f"den{b}")
                ed2 = spool.tile([P, KS], FP32, tag="ed2", name=f"ed2{b}")
                mv4 = spool.tile([P, KS, 2], FP32, tag="mv4", name=f"mv4{b}")
                for m in range(KS):
                    nps = psA.tile([P, D_ATT], FP32, tag="ps")
                    dps = psB.tile([P, 1], FP32, tag="den")
                    for t in range(KS):
                        lw = ET[:, t, P * m : P * (m + 1)]
                        nc.tensor.matmul(
                            nps, lhsT=lw, rhs=Vbf[:, t, :],
                            start=(t == 0), stop=(t == KS - 1),
                        )
                        nc.tensor.matmul(
                            dps, lhsT=lw, rhs=ones_col,
                            start=(t == 0), stop=(t == KS - 1),
                        )
                    nc.vector.tensor_copy(den_sb[:, m : m + 1], dps)
                    nc.vector.scalar_tensor_tensor(
                        out=o4[:, m, :], in0=Vbf[:, m, :],
                        scalar=den_sb[:, m : m + 1], in1=nps,
                        op0=OP.mult, op1=OP.add,
                    )
                    nc.vector.tensor_scalar(
                        ed2[:, m : m + 1], den_sb[:, m : m + 1],
                        den_sb[:, m : m + 1], EPS, OP.mult, OP.mult,
                    )
                    stats = spool.tile([P, 6], FP32, tag="stats")
                    nc.vector.bn_stats(stats, o4[:, m, :])
                    nc.vector.bn_aggr(mv4[:, m, :], stats)
                # batched per-item LN scalars: one ACT hop for sqrt
                arg4 = spool.tile([P, KS], FP32, tag="arg4", name=f"arg4{b}")
                nc.vector.tensor_add(arg4, mv4[:, :, 1], ed2)
                sd4 = spool.tile([P, KS], FP32, tag="sd4", name=f"sd4{b}")
                nc.scalar.activation(sd4, arg4, AF.Sqrt, bias=zero_t)
                rs4 = spool.tile([P, KS], FP32, tag="rs4", name=f"rs4{b}")
                nc.vector.reciprocal(rs4, sd4)
                nmr4 = spool.tile([P, KS], FP32, tag="nmr4", name=f"nmr4{b}")
                nc.vector.tensor_tensor(nmr4, mv4[:, :, 0], rs4, op=OP.mult)
                nc.vector.tensor_scalar(nmr4, nmr4, -1.0, None, OP.mult)
                for m in range(KS):
                    nc.scalar.activation(
                        out=t2[:, m, :],
                        in_=o4[:, m, :],
                        func=AF.Identity,
                        bias=nmr4[:, m : m + 1],
                        scale=rs4[:, m : m + 1],
                    )

                # transpose t2 -> t2T [d, s] (bf16)
                t2T = wpool.tile([P, KD, S], BF16, tag="t2T", name=f"t2T{b}")
                for tt in range(KD):
                    tp = psA.tile([P, S], BF16, tag="ps", name=f"tpt{b}_{tt}")
                    for a in range(KS):
                        nc.tensor.transpose(
                            out=tp[:, P * a : P * (a + 1)],
                            in_=t2[:, a, P * tt : P * (tt + 1)],
                            identity=ident,
                        )
                    nc.scalar.copy(out=t2T[:, tt, :], in_=tp)

                # final: out = t2 @ Wg2 + c
                for m in range(KS):
                    fps = psA.tile([P, D_ATT], FP32, tag="ps")
                    for t in range(KD):
                        nc.tensor.matmul(
                            fps,
                            lhsT=t2T[:, t, P * m : P * (m + 1)],
                            rhs=wg2[:, t, :],
                            start=(t == 0),
                            stop=(t == KD - 1),
                        )
                    out_sb = opool.tile([P, D_ATT], FP32, tag="out_sb")
                    nc.vector.tensor_add(out_sb, fps, cf)
                    nc.sync.dma_start(
                        out=out_d[b, P * m : P * (m + 1), :], in_=out_sb
                    )

            held = {}
            for b in range(n_items + 1):
                if b < n_items:
                    held[b] = front(b)
                if b >= 1:
                    tail(b - 1, *held.pop(b - 1))
    nc.compile()
    return nc


def host_consts(Wq, bq, Wk, bk, Wv, bv, Wf, bf, pos_emb, gamma, beta):
    """One-time host-side weight-layout transforms (input-data independent)."""
    import ml_dtypes

    f32 = np.float32
    bf16 = ml_dtypes.bfloat16
    pos_emb = np.asarray(pos_emb, f32)[:S]
    gamma = np.asarray(gamma, f32)
    beta = np.asarray(beta, f32)
    Wf = np.asarray(Wf, f32)
    wg2 = gamma[:, None] * Wf + np.diag(gamma).astype(f32)
    c_row = beta @ Wf + np.asarray(bf, f32) + beta
    return {
        "wq": np.ascontiguousarray(np.asarray(Wq, f32).astype(bf16)),
        "wk": np.ascontiguousarray(np.asarray(Wk, f32).astype(bf16)),
        "wv": np.ascontiguousarray(np.asarray(Wv, f32).astype(bf16)),
        "wg2": np.ascontiguousarray(wg2.astype(bf16)),
        "petbq": np.ascontiguousarray(pos_emb.T + np.asarray(bq, f32)[:, None]),
        "petbk": np.ascontiguousarray(pos_emb.T + np.asarray(bk, f32)[:, None]),
        "pebv": np.ascontiguousarray(pos_emb + np.asarray(bv, f32)[None, :]),
        "cfull": np.ascontiguousarray(np.broadcast_to(c_row.astype(f32), (P, D_ATT))),
        "iota4": np.ascontiguousarray(
            (np.arange(P, dtype=f32)[:, None] + P * np.arange(KS, dtype=f32)[None, :])
        ),
        "ident": np.eye(P, dtype=f32).astype(bf16),
        "ident32": np.eye(P, dtype=f32),
    }


_prog_cache = {}


def _get_program(n_items):
    if n_items not in _prog_cache:
        _prog_cache[n_items] = build_program(n_items)
    return _prog_cache[n_items]


def kernel(x, mask_start, Wq, bq, Wk, bk, Wv, bv, Wf, bf, pos_emb, gamma, beta):
    global LAST_RESULTS
    x = np.asarray(x, np.float32)
    mask_f = np.asarray(mask_start).astype(np.float32)
    consts = host_consts(Wq, bq, Wk, bk, Wv, bv, Wf, bf, pos_emb, gamma, beta)

    nc = _get_program(BPC)
    in_maps = []
    for c in range(N_CORES):
        m = dict(consts)
        m["x"] = np.ascontiguousarray(x[c * BPC : (c + 1) * BPC])
        m["mstart"] = np.ascontiguousarray(mask_f[c * BPC : (c + 1) * BPC])[None, :]
        in_maps.append(m)

    res = run_bass_kernel_spmd(nc, in_maps, core_ids=list(range(N_CORES)), trace=TRACE)
    LAST_RESULTS = res
    out = np.concatenate([res.results[c]["out"] for c in range(N_CORES)], axis=0)
    return out


# revision 20
# speedup vs baseline: 1.0752x; 1.0752x over previous
"""Trainium2 Bass kernel for masked-attention transformer block.

Computes, per batch item b (B=256, S=512, D_IN=256, D_ATT=512):
    Q = x@Wq + bq + pe;  K = x@Wk + bk + pe;  V = x@Wv + bv + pe
    scores = Q K^T / sqrt(D);  scores[:, k >= mask_start[b]] = -inf
    attn = softmax(scores);  o = attn@V + V;  y = LN(o) * gamma + beta
    out = y@Wf + bf + y

Sharding: data-parallel over batch, 32 items per core across 8 cores.

Strategy (per item):
  - matmuls in bf16 (fp32 is 4 cycles/row on the PE, bf16 is 1);
    accumulation stays fp32 in PSUM; softmax normalization, residuals
    and layernorm statistics stay fp32.
  - x^T via PE transposes (DMA transpose needs 16-bit; done post-cast)
  - Q^T,K^T computed as [d, s] (lhsT=W, rhs=x^T); V natural [s, d]
    (lhsT=x^T, rhs=Wv)
  - scores^T [k, q] = (K^T-slice as lhsT) @ Q^T; key-mask + 1/sqrt(D)
    fused into the Exp activation (per-partition bias/scale) -> E^T
  - numerator = (E^T-slice as lhsT) @ V -> [q, d]; denominator from an
    extra rhs=ones column matmul -> [q, 1] (per-partition)
  - o = num * (1/den) + V in one fused scalar_tensor_tensor DVE op
  - LN stats via bn_stats/bn_aggr; rsqrt on idle GpSimd via pow;
    y_pre = (o - mu) * rs on GpSimd, output directly in bf16
  - gamma/beta folded host-side: Wg2 = diag(gamma') @ Wf + diag(gamma),
    c = beta@Wf + bf + beta, so out = y_pre@Wg2 + c (one matmul + add)
  - y_pre transposed on PE (bf16) for the final matmul
"""

import numpy as np

import concourse.bass as bass
import concourse.tile as tile
from concourse import bacc, mybir
from concourse.bass_utils import run_bass_kernel_spmd

N_CORES = 8
B, S, D_IN, D_ATT = 256, 512, 256, 512
BPC = B // N_CORES
EPS = 1e-5
SCALE = float(1.0 / np.sqrt(D_ATT))
NEG = -30000.0
FP32 = mybir.dt.float32
BF16 = mybir.dt.bfloat16
P = 128
KI = D_IN // P   # 2  k-tiles over input dim
KS = S // P      # 4  tiles over seq
KD = D_ATT // P  # 4  tiles over attention dim

AF = mybir.ActivationFunctionType
OP = mybir.AluOpType

# set by test harness to capture profiling info
TRACE = False
LAST_RESULTS = None


def build_program(n_items):
    nc = bacc.Bacc(None, target_bir_lowering=False, debug=False)

    x_d = nc.dram_tensor("x", [n_items, S, D_IN], FP32, kind="ExternalInput")
    m_d = nc.dram_tensor("mstart", [1, n_items], FP32, kind="ExternalInput")
    wq_d = nc.dram_tensor("wq", [D_IN, D_ATT], BF16, kind="ExternalInput")
    wk_d = nc.dram_tensor("wk", [D_IN, D_ATT], BF16, kind="ExternalInput")
    wv_d = nc.dram_tensor("wv", [D_IN, D_ATT], BF16, kind="ExternalInput")
    wg2_d = nc.dram_tensor("wg2", [D_ATT, D_ATT], BF16, kind="ExternalInput")
    pbq_d = nc.dram_tensor("petbq", [D_ATT, S], FP32, kind="ExternalInput")
    pbk_d = nc.dram_tensor("petbk", [D_ATT, S], FP32, kind="ExternalInput")
    pbv_d = nc.dram_tensor("pebv", [S, D_ATT], FP32, kind="ExternalInput")
    cf_d = nc.dram_tensor("cfull", [P, D_ATT], FP32, kind="ExternalInput")
    io_d = nc.dram_tensor("iota4", [P, KS], FP32, kind="ExternalInput")
    id_d = nc.dram_tensor("ident", [P, P], BF16, kind="ExternalInput")
    id32_d = nc.dram_tensor("ident32", [P, P], FP32, kind="ExternalInput")
    out_d = nc.dram_tensor("out", [n_items, S, D_ATT], FP32, kind="ExternalOutput")

    with tile.TileContext(nc) as tc:
        with (
            tc.tile_pool(name="const", bufs=1) as cpool,
            tc.tile_pool(name="work", bufs=3) as wpool,
            tc.tile_pool(name="outp", bufs=4) as opool,
            tc.tile_pool(name="small", bufs=3) as spool,
            tc.tile_pool(name="psA", bufs=8, space="PSUM") as psA,
        ):
            # ---------------- constants (loaded once) ----------------
            wq = cpool.tile([P, KI, D_ATT], BF16, name="wq_sb")
            nc.sync.dma_start(out=wq, in_=wq_d[:].rearrange("(k p) d -> p k d", p=P))
            wk = cpool.tile([P, KI, D_ATT], BF16, name="wk_sb")
            nc.sync.dma_start(out=wk, in_=wk_d[:].rearrange("(k p) d -> p k d", p=P))
            wv = cpool.tile([P, KI, D_ATT], BF16, name="wv_sb")
            nc.sync.dma_start(out=wv, in_=wv_d[:].rearrange("(k p) d -> p k d", p=P))
            wg2 = cpool.tile([P, KD, D_ATT], BF16, name="wg2_sb")
            nc.sync.dma_start(out=wg2, in_=wg2_d[:].rearrange("(k p) d -> p k d", p=P))
            pbq = cpool.tile([P, KD, S], FP32, name="pbq_sb")
            nc.sync.dma_start(out=pbq, in_=pbq_d[:].rearrange("(m p) s -> p m s", p=P))
            pbk = cpool.tile([P, KD, S], FP32, name="pbk_sb")
            nc.sync.dma_start(out=pbk, in_=pbk_d[:].rearrange("(m p) s -> p m s", p=P))
            pbv = cpool.tile([P, KS, D_ATT], FP32, name="pbv_sb")
            nc.sync.dma_start(out=pbv, in_=pbv_d[:].rearrange("(m p) d -> p m d", p=P))
            cf = cpool.tile([P, D_ATT], FP32, name="cf_sb")
            nc.sync.dma_start(out=cf, in_=cf_d[:])
            iota = cpool.tile([P, KS], FP32, name="iota_sb")
            nc.sync.dma_start(out=iota, in_=io_d[:])
            ident = cpool.tile([P, P], BF16, name="ident_sb")
            nc.sync.dma_start(out=ident, in_=id_d[:])
            ident32 = cpool.tile([P, P], FP32, name="ident32_sb")
            nc.sync.dma_start(out=ident32, in_=id32_d[:])

            ones_col = cpool.tile([P, 1], BF16, name="ones_col")
            nc.vector.memset(ones_col, 1.0)
            eps_t = cpool.tile([P, 1], FP32, name="eps_t")
            nc.vector.memset(eps_t, EPS)
            zero_t = cpool.tile([P, 1], FP32, name="zero_t")
            nc.vector.memset(zero_t, 0.0)

            # broadcast mask starts to all 128 partitions on GpSimd
            m_row = cpool.tile([1, n_items], FP32, name="m_row")
            nc.sync.dma_start(out=m_row, in_=m_d[:])
            m_bc = cpool.tile([P, n_items], FP32, name="m_bc")
            nc.gpsimd.partition_broadcast(m_bc, m_row)

            # ---------------- per-item pipeline ----------------
            # Software-pipelined emission: each engine's (in-order) stream
            # alternates FRONT(b) and TAIL(b-1) so there is always work from
            # two items in flight and the PE never idles past the HAM window.

            def front(b):
                x_nat = wpool.tile([P, KS, D_IN], FP32, tag="xnat", name=f"xnat{b}")
                nc.sync.dma_start(
                    out=x_nat, in_=x_d[b].rearrange("(t p) i -> p t i", p=P)
                )
                # additive key-mask column: 0 where k < m_b else -30000
                maskb = spool.tile([P, KS], FP32, tag="maskb", name=f"maskb{b}")
                nc.vector.tensor_scalar(
                    maskb, iota, m_bc[:, b : b + 1], NEG, OP.is_ge, OP.mult
                )

                # x^T via fp32 PE transposes; bf16 cast fused into psum copy
                xT = wpool.tile([P, KI, S], BF16, tag="xT", name=f"xT{b}")
                for ii in range(KI):
                    tp = psA.tile([P, S], FP32, tag="ps", name=f"tpx{b}_{ii}")
                    for ss in range(KS):
                        nc.tensor.transpose(
                            out=tp[:, P * ss : P * (ss + 1)],
                            in_=x_nat[:, ss, P * ii : P * (ii + 1)],
                            identity=ident32,
                        )
                    nc.scalar.copy(out=xT[:, ii, :], in_=tp)

                # projections: Q^T, K^T as [d, s] bf16; V natural [s, d] fp32
                QT = wpool.tile([P, KD, S], BF16, tag="QT", name=f"QT{b}")
                KTt = wpool.tile([P, KD, S], BF16, tag="KTt", name=f"KTt{b}")
                for dst, w, pb in ((QT, wq, pbq), (KTt, wk, pbk)):
                    for m in range(KD):
                        ps = psA.tile([P, S], FP32, tag="ps")
                        for k in range(KI):
                            nc.tensor.matmul(
                                ps,
                                lhsT=w[:, k, P * m : P * (m + 1)],
                                rhs=xT[:, k, :],
                                start=(k == 0),
                                stop=(k == KI - 1),
                            )
                        nc.vector.tensor_add(dst[:, m, :], ps, pb[:, m, :])
                Vbf = wpool.tile([P, KS, D_ATT], BF16, tag="Vbf", name=f"Vbf{b}")
                for m in range(KS):
                    ps = psA.tile([P, D_ATT], FP32, tag="ps")
                    for k in range(KI):
                        nc.tensor.matmul(
                            ps,
                            lhsT=xT[:, k, P * m : P * (m + 1)],
                            rhs=wv[:, k, :],
                            start=(k == 0),
                            stop=(k == KI - 1),
                        )
                    nc.vector.tensor_add(Vbf[:, m, :], ps, pbv[:, m, :])

                # scores^T [k, q] + fused mask/scale/exp -> E^T (bf16)
                ET = wpool.tile([P, KS, S], BF16, tag="ET", name=f"ET{b}")
                for t in range(KS):
                    ps = psA.tile([P, S], FP32, tag="ps")
                    for m in range(KD):
                        nc.tensor.matmul(
                            ps,
                            lhsT=KTt[:, m, P * t : P * (t + 1)],
                            rhs=QT[:, m, :],
                            start=(m == 0),
                            stop=(m == KD - 1),
                        )
                    nc.scalar.activation(
                        out=ET[:, t, :],
                        in_=ps,
                        func=AF.Exp,
                        bias=maskb[:, t : t + 1],
                        scale=SCALE,
                    )
                return ET, Vbf

            def tail1(b, ET, Vbf):
                # attention output; layernorm of o/den == layernorm of o
                # (row-scale invariance), so no reciprocal: o'' = den*V + num
                # and eps is scaled by den^2 inside the sqrt.
                t2 = wpool.tile([P, KS, D_ATT], BF16, tag="t2", name=f"t2{b}")
                o4 = wpool.tile([P, KS, D_ATT], FP32, tag="o4", name=f"o4{b}")
                den_sb = spool.tile([P, KS], FP32, tag="den_sb", name=f"den{b}")
                ed2 = spool.tile([P, KS], FP32, tag="ed2", name=f"ed2{b}")
                mv4 = spool.tile([P, KS, 2], FP32, tag="mv4", name=f"mv4{b}")
                for m in range(KS):
                    nps = psA.tile([P, D_ATT], FP32, tag="ps")
                    dps = psA.tile([P, 1], FP32, tag="ps")
                    for t in range(KS):
                        lw = ET[:, t, P * m : P * (m + 1)]
                        nc.tensor.matmul(
                            nps, lhsT=lw, rhs=Vbf[:, t, :],
                            start=(t == 0), stop=(t == KS - 1),
                        )
                        nc.tensor.matmul(
                            dps, lhsT=lw, rhs=ones_col,
                            start=(t == 0), stop=(t == KS - 1),
                        )
                    nc.vector.tensor_copy(den_sb[:, m : m + 1], dps)
                    nc.vector.scalar_tensor_tensor(
                        out=o4[:, m, :], in0=Vbf[:, m, :],
                        scalar=den_sb[:, m : m + 1], in1=nps,
                        op0=OP.mult, op1=OP.add,
                    )
                    nc.vector.tensor_scalar(
                        ed2[:, m : m + 1], den_sb[:, m : m + 1],
                        den_sb[:, m : m + 1], EPS, OP.mult, OP.mult,
                    )
                    stats = spool.tile([P, 6], FP32, tag="stats")
                    nc.vector.bn_stats(stats, o4[:, m, :])
                    nc.vector.bn_aggr(mv4[:, m, :], stats)
                # batched per-item LN scalars: one ACT hop for sqrt
                arg4 = spool.tile([P, KS], FP32, tag="arg4", name=f"arg4{b}")
                nc.vector.tensor_add(arg4, mv4[:, :, 1], ed2)
                sd4 = spool.tile([P, KS], FP32, tag="sd4", name=f"sd4{b}")
                nc.scalar.activation(sd4, arg4, AF.Sqrt, bias=zero_t)
                rs4 = spool.tile([P, KS], FP32, tag="rs4", name=f"rs4{b}")
                nc.vector.reciprocal(rs4, sd4)
                nmr4 = spool.tile([P, KS], FP32, tag="nmr4", name=f"nmr4{b}")
                nc.vector.tensor_tensor(nmr4, mv4[:, :, 0], rs4, op=OP.mult)
                nc.vector.tensor_scalar(nmr4, nmr4, -1.0, None, OP.mult)
                for m in range(KS):
                    nc.scalar.activation(
                        out=t2[:, m, :],
                        in_=o4[:, m, :],
                        func=AF.Identity,
                        bias=nmr4[:, m : m + 1],
                        scale=rs4[:, m : m + 1],
                    )

                # transpose t2 -> t2T [d, s] (bf16 PE transposes)
                t2T = wpool.tile([P, KD, S], BF16, tag="t2T", name=f"t2T{b}")
                for tt in range(KD):
                    tp = psA.tile([P, S], BF16, tag="ps", name=f"tpt{b}_{tt}")
                    for a in range(KS):
                        nc.tensor.transpose(
                            out=tp[:, P * a : P * (a + 1)],
                            in_=t2[:, a, P * tt : P * (tt + 1)],
                            identity=ident,
                        )
                    nc.scalar.copy(out=t2T[:, tt, :], in_=tp)
                return t2T

            def tail2(b, t2T):
                # final: out = t2 @ Wg2 + c
                for m in range(KS):
                    fps = psA.tile([P, D_ATT], FP32, tag="ps")
                    for t in range(KD):
                        nc.tensor.matmul(
                            fps,
                            lhsT=t2T[:, t, P * m : P * (m + 1)],
                            rhs=wg2[:, t, :],
                            start=(t == 0),
                            stop=(t == KD - 1),
                        )
                    out_sb = opool.tile([P, D_ATT], FP32, tag="out_sb")
                    nc.vector.tensor_add(out_sb, fps, cf)
                    nc.sync.dma_start(
                        out=out_d[b, P * m : P * (m + 1), :], in_=out_sb
                    )

            held = {}
            held2 = {}
            for b in range(n_items + 2):
                if b < n_items:
                    held[b] = front(b)
                if b >= 1 and b - 1 < n_items:
                    held2[b - 1] = tail1(b - 1, *held.pop(b - 1))
                if b >= 2:
                    tail2(b - 2, held2.pop(b - 2))
    nc.compile()
    return nc


def host_consts(Wq, bq, Wk, bk, Wv, bv, Wf, bf, pos_emb, gamma, beta):
    """One-time host-side weight-layout transforms (input-data independent)."""
    import ml_dtypes

    f32 = np.float32
    bf16 = ml_dtypes.bfloat16
    pos_emb = np.asarray(pos_emb, f32)[:S]
    gamma = np.asarray(gamma, f32)
    beta = np.asarray(beta, f32)
    Wf = np.asarray(Wf, f32)
    wg2 = gamma[:, None] * Wf + np.diag(gamma).astype(f32)
    c_row = beta @ Wf + np.asarray(bf, f32) + beta
    return {
        "wq": np.ascontiguousarray(np.asarray(Wq, f32).astype(bf16)),
        "wk": np.ascontiguousarray(np.asarray(Wk, f32).astype(bf16)),
        "wv": np.ascontiguousarray(np.asarray(Wv, f32).astype(bf16)),
        "wg2": np.ascontiguousarray(wg2.astype(bf16)),
        "petbq": np.ascontiguousarray(pos_emb.T + np.asarray(bq, f32)[:, None]),
        "petbk": np.ascontiguousarray(pos_emb.T + np.asarray(bk, f32)[:, None]),
        "pebv": np.ascontiguousarray(pos_emb + np.asarray(bv, f32)[None, :]),
        "cfull": np.ascontiguousarray(np.broadcast_to(c_row.astype(f32), (P, D_ATT))),
        "iota4": np.ascontiguousarray(
            (np.arange(P, dtype=f32)[:, None] + P * np.arange(KS, dtype=f32)[None, :])
        ),
        "ident": np.eye(P, dtype=f32).astype(bf16),
        "ident32": np.eye(P, dtype=f32),
    }


_prog_cache = {}


def _get_program(n_items):
    if n_items not in _prog_cache:
        _prog_cache[n_items] = build_program(n_items)
    return _prog_cache[n_items]


def kernel(x, mask_start, Wq, bq, Wk, bk, Wv, bv, Wf, bf, pos_emb, gamma, beta):
    global LAST_RESULTS
    x = np.asarray(x, np.float32)
    mask_f = np.asarray(mask_start).astype(np.float32)
    consts = host_consts(Wq, bq, Wk, bk, Wv, bv, Wf, bf, pos_emb, gamma, beta)

    nc = _get_program(BPC)
    in_maps = []
    for c in range(N_CORES):
        m = dict(consts)
        m["x"] = np.ascontiguousarray(x[c * BPC : (c + 1) * BPC])
        m["mstart"] = np.ascontiguousarray(mask_f[c * BPC : (c + 1) * BPC])[None, :]
        in_maps.append(m)

    res = run_bass_kernel_spmd(nc, in_maps, core_ids=list(range(N_CORES)), trace=TRACE)
    LAST_RESULTS = res
    out = np.concatenate([res.results[c]["out"] for c in range(N_CORES)], axis=0)
    return out


# revision 21
# speedup vs baseline: 1.4920x; 1.3876x over previous
"""Trainium2 Bass kernel for masked-attention transformer block.

Computes, per batch item b (B=256, S=512, D_IN=256, D_ATT=512):
    Q = x@Wq + bq + pe;  K = x@Wk + bk + pe;  V = x@Wv + bv + pe
    scores = Q K^T / sqrt(D);  scores[:, k >= mask_start[b]] = -inf
    attn = softmax(scores);  o = attn@V + V;  y = LN(o) * gamma + beta
    out = y@Wf + bf + y

Sharding: data-parallel over batch, 32 items per core across 8 cores.

Strategy (per item):
  - matmuls in bf16 (fp32 is 4 cycles/row on the PE, bf16 is 1);
    accumulation stays fp32 in PSUM; softmax normalization, residuals
    and layernorm statistics stay fp32.
  - x^T via PE transposes (DMA transpose needs 16-bit; done post-cast)
  - Q^T,K^T computed as [d, s] (lhsT=W, rhs=x^T); V natural [s, d]
    (lhsT=x^T, rhs=Wv)
  - scores^T [k, q] = (K^T-slice as lhsT) @ Q^T; key-mask + 1/sqrt(D)
    fused into the Exp activation (per-partition bias/scale) -> E^T
  - numerator = (E^T-slice as lhsT) @ V -> [q, d]; denominator from an
    extra rhs=ones column matmul -> [q, 1] (per-partition)
  - o = num * (1/den) + V in one fused scalar_tensor_tensor DVE op
  - LN stats via bn_stats/bn_aggr; rsqrt on idle GpSimd via pow;
    y_pre = (o - mu) * rs on GpSimd, output directly in bf16
  - gamma/beta folded host-side: Wg2 = diag(gamma') @ Wf + diag(gamma),
    c = beta@Wf + bf + beta, so out = y_pre@Wg2 + c (one matmul + add)
  - y_pre transposed on PE (bf16) for the final matmul
"""

import numpy as np

import concourse.bass as bass
import concourse.tile as tile
from concourse import bacc, mybir
from concourse.bass_utils import run_bass_kernel_spmd

N_CORES = 8
B, S, D_IN, D_ATT = 256, 512, 256, 512
BPC = B // N_CORES
EPS = 1e-5
SCALE = float(1.0 / np.sqrt(D_ATT))
NEG = -30000.0
FP32 = mybir.dt.float32
BF16 = mybir.dt.bfloat16
P = 128
KI = D_IN // P   # 2  k-tiles over input dim
KS = S // P      # 4  tiles over seq
KD = D_ATT // P  # 4  tiles over attention dim

AF = mybir.ActivationFunctionType
OP = mybir.AluOpType

# set by test harness to capture profiling info
TRACE = False
LAST_RESULTS = None


def build_program(n_items):
    nc = bacc.Bacc(None, target_bir_lowering=False, debug=False)

    x_d = nc.dram_tensor("x", [n_items, S, D_IN], FP32, kind="ExternalInput")
    m_d = nc.dram_tensor("mstart", [1, n_items], FP32, kind="ExternalInput")
    wq_d = nc.dram_tensor("wq", [D_IN, D_ATT], BF16, kind="ExternalInput")
    wk_d = nc.dram_tensor("wk", [D_IN, D_ATT], BF16, kind="ExternalInput")
    wv_d = nc.dram_tensor("wv", [D_IN, D_ATT], BF16, kind="ExternalInput")
    wg2_d = nc.dram_tensor("wg2", [D_ATT, D_ATT], BF16, kind="ExternalInput")
    pbq_d = nc.dram_tensor("petbq", [D_ATT, S], FP32, kind="ExternalInput")
    pbk_d = nc.dram_tensor("petbk", [D_ATT, S], FP32, kind="ExternalInput")
    pbv_d = nc.dram_tensor("pebv", [S, D_ATT], FP32, kind="ExternalInput")
    cf_d = nc.dram_tensor("cfull", [P, D_ATT], FP32, kind="ExternalInput")
    io_d = nc.dram_tensor("iota4", [P, KS], FP32, kind="ExternalInput")
    id_d = nc.dram_tensor("ident", [P, P], BF16, kind="ExternalInput")
    id32_d = nc.dram_tensor("ident32", [P, P], FP32, kind="ExternalInput")
    out_d = nc.dram_tensor("out", [n_items, S, D_ATT], FP32, kind="ExternalOutput")

    with tile.TileContext(nc) as tc:
        with (
            tc.tile_pool(name="const", bufs=1) as cpool,
            tc.tile_pool(name="work", bufs=3) as wpool,
            tc.tile_pool(name="outp", bufs=4) as opool,
            tc.tile_pool(name="small", bufs=3) as spool,
            tc.tile_pool(name="psA", bufs=8, space="PSUM") as psA,
        ):
            # ---------------- constants (loaded once) ----------------
            wq = cpool.tile([P, KI, D_ATT], BF16, name="wq_sb")
            nc.sync.dma_start(out=wq, in_=wq_d[:].rearrange("(k p) d -> p k d", p=P))
            wk = cpool.tile([P, KI, D_ATT], BF16, name="wk_sb")
            nc.sync.dma_start(out=wk, in_=wk_d[:].rearrange("(k p) d -> p k d", p=P))
            wv = cpool.tile([P, KI, D_ATT], BF16, name="wv_sb")
            nc.sync.dma_start(out=wv, in_=wv_d[:].rearrange("(k p) d -> p k d", p=P))
            wg2 = cpool.tile([P, KD, D_ATT], BF16, name="wg2_sb")
            nc.sync.dma_start(out=wg2, in_=wg2_d[:].rearrange("(k p) d -> p k d", p=P))
            pbq = cpool.tile([P, KD, S], FP32, name="pbq_sb")
            nc.sync.dma_start(out=pbq, in_=pbq_d[:].rearrange("(m p) s -> p m s", p=P))
            pbk = cpool.tile([P, KD, S], FP32, name="pbk_sb")
            nc.sync.dma_start(out=pbk, in_=pbk_d[:].rearrange("(m p) s -> p m s", p=P))
            pbv = cpool.tile([P, KS, D_ATT], FP32, name="pbv_sb")
            nc.sync.dma_start(out=pbv, in_=pbv_d[:].rearrange("(m p) d -> p m d", p=P))
            cf = cpool.tile([P, D_ATT], FP32, name="cf_sb")
            nc.sync.dma_start(out=cf, in_=cf_d[:])
            iota = cpool.tile([P, KS], FP32, name="iota_sb")
            nc.sync.dma_start(out=iota, in_=io_d[:])
            ident = cpool.tile([P, P], BF16, name="ident_sb")
            nc.sync.dma_start(out=ident, in_=id_d[:])
            ident32 = cpool.tile([P, P], FP32, name="ident32_sb")
            nc.sync.dma_start(out=ident32, in_=id32_d[:])

            ones_col = cpool.tile([P, 1], BF16, name="ones_col")
            nc.vector.memset(ones_col, 1.0)
            eps_t = cpool.tile([P, 1], FP32, name="eps_t")
            nc.vector.memset(eps_t, EPS)
            zero_t = cpool.tile([P, 1], FP32, name="zero_t")
            nc.vector.memset(zero_t, 0.0)

            # broadcast mask starts to all 128 partitions on GpSimd
            m_row = cpool.tile([1, n_items], FP32, name="m_row")
            nc.sync.dma_start(out=m_row, in_=m_d[:])
            m_bc = cpool.tile([P, n_items], FP32, name="m_bc")
            nc.gpsimd.partition_broadcast(m_bc, m_row)

            # ---------------- per-item pipeline ----------------
            # Software-pipelined emission: each engine's (in-order) stream
            # alternates FRONT(b) and TAIL(b-1) so there is always work from
            # two items in flight and the PE never idles past the HAM window.

            def front(b):
                x_nat = wpool.tile([P, KS, D_IN], FP32, tag="xnat", name=f"xnat{b}")
                nc.sync.dma_start(
                    out=x_nat, in_=x_d[b].rearrange("(t p) i -> p t i", p=P)
                )
                # additive key-mask column: 0 where k < m_b else -30000
                maskb = spool.tile([P, KS], FP32, tag="maskb", name=f"maskb{b}")
                nc.vector.tensor_scalar(
                    maskb, iota, m_bc[:, b : b + 1], NEG, OP.is_ge, OP.mult
                )

                # x^T via fp32 PE transposes; bf16 cast fused into psum copy
                xT = wpool.tile([P, KI, S], BF16, tag="xT", name=f"xT{b}")
                for ii in range(KI):
                    tp = psA.tile([P, S], FP32, tag="ps", name=f"tpx{b}_{ii}")
                    for ss in range(KS):
                        nc.tensor.transpose(
                            out=tp[:, P * ss : P * (ss + 1)],
                            in_=x_nat[:, ss, P * ii : P * (ii + 1)],
                            identity=ident32,
                        )
                    nc.scalar.copy(out=xT[:, ii, :], in_=tp)

                # projections: Q^T, K^T as [d, s] bf16; V natural [s, d] fp32
                QT = wpool.tile([P, KD, S], BF16, tag="QT", name=f"QT{b}")
                KTt = wpool.tile([P, KD, S], BF16, tag="KTt", name=f"KTt{b}")
                for dst, w, pb in ((QT, wq, pbq), (KTt, wk, pbk)):
                    for m in range(KD):
                        ps = psA.tile([P, S], FP32, tag="ps")
                        for k in range(KI):
                            nc.tensor.matmul(
                                ps,
                                lhsT=w[:, k, P * m : P * (m + 1)],
                                rhs=xT[:, k, :],
                                start=(k == 0),
                                stop=(k == KI - 1),
                            )
                        nc.vector.tensor_add(dst[:, m, :], ps, pb[:, m, :])
                Vbf = wpool.tile([P, KS, D_ATT], BF16, tag="Vbf", name=f"Vbf{b}")
                for m in range(KS):
                    ps = psA.tile([P, D_ATT], FP32, tag="ps")
                    for k in range(KI):
                        nc.tensor.matmul(
                            ps,
                            lhsT=xT[:, k, P * m : P * (m + 1)],
                            rhs=wv[:, k, :],
                            start=(k == 0),
                            stop=(k == KI - 1),
                        )
                    nc.vector.tensor_add(Vbf[:, m, :], ps, pbv[:, m, :])

                # scores^T [k, q] + fused mask/scale/exp -> E^T (bf16)
                ET = wpool.tile([P, KS, S], BF16, tag="ET", name=f"ET{b}")
                for t in range(KS):
                    ps = psA.tile([P, S], FP32, tag="ps")
                    for m in range(KD):
                        nc.tensor.matmul(
                            ps,
                            lhsT=KTt[:, m, P * t : P * (t + 1)],
                            rhs=QT[:, m, :],
                            start=(m == 0),
                            stop=(m == KD - 1),
                        )
                    nc.scalar.activation(
                        out=ET[:, t, :],
                        in_=ps,
                        func=AF.Exp,
                        bias=maskb[:, t : t + 1],
                        scale=SCALE,
                    )
                return ET, Vbf

            def tail1(b, ET, Vbf):
                # attention output; layernorm of o/den == layernorm of o
                # (row-scale invariance), so no reciprocal: o'' = den*V + num
                # and eps is scaled by den^2 inside the sqrt.
                t2 = wpool.tile([P, KS, D_ATT], BF16, tag="t2", name=f"t2{b}")
                o4 = wpool.tile([P, KS, D_ATT], FP32, tag="o4", name=f"o4{b}")
                den_sb = spool.tile([P, KS], FP32, tag="den_sb", name=f"den{b}")
                ed2 = spool.tile([P, KS], FP32, tag="ed2", name=f"ed2{b}")
                mv4 = spool.tile([P, KS, 2], FP32, tag="mv4", name=f"mv4{b}")
                for m in range(KS):
                    nps = psA.tile([P, D_ATT], FP32, tag="ps")
                    dps = psA.tile([P, 1], FP32, tag="ps")
                    for t in range(KS):
                        lw = ET[:, t, P * m : P * (m + 1)]
                        nc.tensor.matmul(
                            nps, lhsT=lw, rhs=Vbf[:, t, :],
                            start=(t == 0), stop=(t == KS - 1),
                        )
                        nc.tensor.matmul(
                            dps, lhsT=lw, rhs=ones_col,
                            start=(t == 0), stop=(t == KS - 1),
                        )
                    nc.vector.tensor_copy(den_sb[:, m : m + 1], dps)
                    nc.vector.scalar_tensor_tensor(
                        out=o4[:, m, :], in0=Vbf[:, m, :],
                        scalar=den_sb[:, m : m + 1], in1=nps,
                        op0=OP.mult, op1=OP.add,
                    )
                    nc.vector.tensor_scalar(
                        ed2[:, m : m + 1], den_sb[:, m : m + 1],
                        den_sb[:, m : m + 1], EPS, OP.mult, OP.mult,
                    )
                    stats = spool.tile([P, 6], FP32, tag="stats")
                    nc.vector.bn_stats(stats, o4[:, m, :])
                    nc.vector.bn_aggr(mv4[:, m, :], stats)
                # batched per-item LN scalars: one ACT hop for sqrt
                arg4 = spool.tile([P, KS], FP32, tag="arg4", name=f"arg4{b}")
                nc.vector.tensor_add(arg4, mv4[:, :, 1], ed2)
                sd4 = spool.tile([P, KS], FP32, tag="sd4", name=f"sd4{b}")
                nc.scalar.activation(sd4, arg4, AF.Sqrt, bias=zero_t)
                rs4 = spool.tile([P, KS], FP32, tag="rs4", name=f"rs4{b}")
                nc.vector.reciprocal(rs4, sd4)
                nmr4 = spool.tile([P, KS], FP32, tag="nmr4", name=f"nmr4{b}")
                nc.vector.tensor_tensor(nmr4, mv4[:, :, 0], rs4, op=OP.mult)
                nc.vector.tensor_scalar(nmr4, nmr4, -1.0, None, OP.mult)
                for m in range(KS):
                    nc.scalar.activation(
                        out=t2[:, m, :],
                        in_=o4[:, m, :],
                        func=AF.Identity,
                        bias=nmr4[:, m : m + 1],
                        scale=rs4[:, m : m + 1],
                    )

                return t2

            def tail2(b, t2):
                # transpose t2 -> t2T [d, s] (bf16 PE transposes)
                t2T = wpool.tile([P, KD, S], BF16, tag="t2T", name=f"t2T{b}")
                for tt in range(KD):
                    tp = psA.tile([P, S], BF16, tag="ps", name=f"tpt{b}_{tt}")
                    for a in range(KS):
                        nc.tensor.transpose(
                            out=tp[:, P * a : P * (a + 1)],
                            in_=t2[:, a, P * tt : P * (tt + 1)],
                            identity=ident,
                        )
                    nc.scalar.copy(out=t2T[:, tt, :], in_=tp)

                # final: out = t2 @ Wg2 + c
                for m in range(KS):
                    fps = psA.tile([P, D_ATT], FP32, tag="ps")
                    for t in range(KD):
                        nc.tensor.matmul(
                            fps,
                            lhsT=t2T[:, t, P * m : P * (m + 1)],
                            rhs=wg2[:, t, :],
                            start=(t == 0),
                            stop=(t == KD - 1),
                        )
                    out_sb = opool.tile([P, D_ATT], FP32, tag="out_sb")
                    nc.vector.tensor_add(out_sb, fps, cf)
                    nc.sync.dma_start(
                        out=out_d[b, P * m : P * (m + 1), :], in_=out_sb
                    )

            held = {}
            held2 = {}
            for b in range(n_items + 2):
                if b < n_items:
                    held[b] = front(b)
                if b >= 1 and b - 1 < n_items:
                    held2[b - 1] = tail1(b - 1, *held.pop(b - 1))
                if b >= 2:
                    tail2(b - 2, held2.pop(b - 2))
    nc.compile()
    return nc


def host_consts(Wq, bq, Wk, bk, Wv, bv, Wf, bf, pos_emb, gamma, beta):
    """One-time host-side weight-layout transforms (input-data independent)."""
    import ml_dtypes

    f32 = np.float32
    bf16 = ml_dtypes.bfloat16
    pos_emb = np.asarray(pos_emb, f32)[:S]
    gamma = np.asarray(gamma, f32)
    beta = np.asarray(beta, f32)
    Wf = np.asarray(Wf, f32)
    wg2 = gamma[:, None] * Wf + np.diag(gamma).astype(f32)
    c_row = beta @ Wf + np.asarray(bf, f32) + beta
    return {
        "wq": np.ascontiguousarray(np.asarray(Wq, f32).astype(bf16)),
        "wk": np.ascontiguousarray(np.asarray(Wk, f32).astype(bf16)),
        "wv": np.ascontiguousarray(np.asarray(Wv, f32).astype(bf16)),
        "wg2": np.ascontiguousarray(wg2.astype(bf16)),
        "petbq": np.ascontiguousarray(pos_emb.T + np.asarray(bq, f32)[:, None]),
        "petbk": np.ascontiguousarray(pos_emb.T + np.asarray(bk, f32)[:, None]),
        "pebv": np.ascontiguousarray(pos_emb + np.asarray(bv, f32)[None, :]),
        "cfull": np.ascontiguousarray(np.broadcast_to(c_row.astype(f32), (P, D_ATT))),
        "iota4": np.ascontiguousarray(
            (np.arange(P, dtype=f32)[:, None] + P * np.arange(KS, dtype=f32)[None, :])
        ),
        "ident": np.eye(P, dtype=f32).astype(bf16),
        "ident32": np.eye(P, dtype=f32),
    }


_prog_cache = {}


def _get_program(n_items):
    if n_items not in _prog_cache:
        _prog_cache[n_items] = build_program(n_items)
    return _prog_cache[n_items]


def kernel(x, mask_start, Wq, bq, Wk, bk, Wv, bv, Wf, bf, pos_emb, gamma, beta):
    global LAST_RESULTS
    x = np.asarray(x, np.float32)
    mask_f = np.asarray(mask_start).astype(np.float32)
    consts = host_consts(Wq, bq, Wk, bk, Wv, bv, Wf, bf, pos_emb, gamma, beta)

    nc = _get_program(BPC)
    in_maps = []
    for c in range(N_CORES):
        m = dict(consts)
        m["x"] = np.ascontiguousarray(x[c * BPC : (c + 1) * BPC])
        m["mstart"] = np.ascontiguousarray(mask_f[c * BPC : (c + 1) * BPC])[None, :]
        in_maps.append(m)

    res = run_bass_kernel_spmd(nc, in_maps, core_ids=list(range(N_CORES)), trace=TRACE)
    LAST_RESULTS = res
    out = np.concatenate([res.results[c]["out"] for c in range(N_CORES)], axis=0)
    return out


# revision 22
# speedup vs baseline: 1.5528x; 1.0408x over previous
"""Trainium2 Bass kernel for masked-attention transformer block.

Computes, per batch item b (B=256, S=512, D_IN=256, D_ATT=512):
    Q = x@Wq + bq + pe;  K = x@Wk + bk + pe;  V = x@Wv + bv + pe
    scores = Q K^T / sqrt(D);  scores[:, k >= mask_start[b]] = -inf
    attn = softmax(scores);  o = attn@V + V;  y = LN(o) * gamma + beta
    out = y@Wf + bf + y

Sharding: data-parallel over batch, 32 items per core across 8 cores.

Strategy (per item):
  - matmuls in bf16 (fp32 is 4 cycles/row on the PE, bf16 is 1);
    accumulation stays fp32 in PSUM; softmax normalization, residuals
    and layernorm statistics stay fp32.
  - x^T via PE transposes (DMA transpose needs 16-bit; done post-cast)
  - Q^T,K^T computed as [d, s] (lhsT=W, rhs=x^T); V natural [s, d]
    (lhsT=x^T, rhs=Wv)
  - scores^T [k, q] = (K^T-slice as lhsT) @ Q^T; key-mask + 1/sqrt(D)
    fused into the Exp activation (per-partition bias/scale) -> E^T
  - numerator = (E^T-slice as lhsT) @ V -> [q, d]; denominator from an
    extra rhs=ones column matmul -> [q, 1] (per-partition)
  - o = num * (1/den) + V in one fused scalar_tensor_tensor DVE op
  - LN stats via bn_stats/bn_aggr; rsqrt on idle GpSimd via pow;
    y_pre = (o - mu) * rs on GpSimd, output directly in bf16
  - gamma/beta folded host-side: Wg2 = diag(gamma') @ Wf + diag(gamma),
    c = beta@Wf + bf + beta, so out = y_pre@Wg2 + c (one matmul + add)
  - y_pre transposed on PE (bf16) for the final matmul
"""

import numpy as np

import concourse.bass as bass
import concourse.tile as tile
from concourse import bacc, mybir
from concourse.bass_utils import run_bass_kernel_spmd

N_CORES = 8
B, S, D_IN, D_ATT = 256, 512, 256, 512
BPC = B // N_CORES
EPS = 1e-5
SCALE = float(1.0 / np.sqrt(D_ATT))
NEG = -30000.0
FP32 = mybir.dt.float32
BF16 = mybir.dt.bfloat16
P = 128
KI = D_IN // P   # 2  k-tiles over input dim
KS = S // P      # 4  tiles over seq
KD = D_ATT // P  # 4  tiles over attention dim

AF = mybir.ActivationFunctionType
OP = mybir.AluOpType

# set by test harness to capture profiling info
TRACE = False
LAST_RESULTS = None


def build_program(n_items):
    nc = bacc.Bacc(None, target_bir_lowering=False, debug=False)

    x_d = nc.dram_tensor("x", [n_items, S, D_IN], FP32, kind="ExternalInput")
    m_d = nc.dram_tensor("mstart", [1, n_items], FP32, kind="ExternalInput")
    wq_d = nc.dram_tensor("wq", [D_IN, D_ATT], BF16, kind="ExternalInput")
    wk_d = nc.dram_tensor("wk", [D_IN, D_ATT], BF16, kind="ExternalInput")
    wv_d = nc.dram_tensor("wv", [D_IN, D_ATT], BF16, kind="ExternalInput")
    wg2_d = nc.dram_tensor("wg2", [D_ATT, D_ATT], BF16, kind="ExternalInput")
    pbq_d = nc.dram_tensor("petbq", [D_ATT, S], FP32, kind="ExternalInput")
    pbk_d = nc.dram_tensor("petbk", [D_ATT, S], FP32, kind="ExternalInput")
    pbv_d = nc.dram_tensor("pebv", [S, D_ATT], FP32, kind="ExternalInput")
    cf_d = nc.dram_tensor("cfull", [P, D_ATT], FP32, kind="ExternalInput")
    io_d = nc.dram_tensor("iota4", [P, KS], FP32, kind="ExternalInput")
    id_d = nc.dram_tensor("ident", [P, P], BF16, kind="ExternalInput")
    out_d = nc.dram_tensor("out", [n_items, S, D_ATT], FP32, kind="ExternalOutput")

    with tile.TileContext(nc) as tc:
        with (
            tc.tile_pool(name="const", bufs=1) as cpool,
            tc.tile_pool(name="work", bufs=3) as wpool,
            tc.tile_pool(name="outp", bufs=4) as opool,
            tc.tile_pool(name="small", bufs=4) as spool,
            tc.tile_pool(name="psA", bufs=8, space="PSUM") as psA,
        ):
            # ---------------- constants (loaded once) ----------------
            # prefetch the first item's activations before the big consts so
            # the PE can start transposing immediately
            x0 = wpool.tile([P, KS, D_IN], FP32, tag="xnat", name="xnat0")
            nc.sync.dma_start(out=x0, in_=x_d[0].rearrange("(t p) i -> p t i", p=P))
            ident = cpool.tile([P, P], BF16, name="ident_sb")
            nc.sync.dma_start(out=ident, in_=id_d[:])
            wq = cpool.tile([P, KI, D_ATT], BF16, name="wq_sb")
            nc.sync.dma_start(out=wq, in_=wq_d[:].rearrange("(k p) d -> p k d", p=P))
            wk = cpool.tile([P, KI, D_ATT], BF16, name="wk_sb")
            nc.sync.dma_start(out=wk, in_=wk_d[:].rearrange("(k p) d -> p k d", p=P))
            wv = cpool.tile([P, KI, D_ATT], BF16, name="wv_sb")
            nc.sync.dma_start(out=wv, in_=wv_d[:].rearrange("(k p) d -> p k d", p=P))
            wg2 = cpool.tile([P, KD, D_ATT], BF16, name="wg2_sb")
            nc.sync.dma_start(out=wg2, in_=wg2_d[:].rearrange("(k p) d -> p k d", p=P))
            pbq = cpool.tile([P, KD, S], FP32, name="pbq_sb")
            nc.sync.dma_start(out=pbq, in_=pbq_d[:].rearrange("(m p) s -> p m s", p=P))
            pbk = cpool.tile([P, KD, S], FP32, name="pbk_sb")
            nc.sync.dma_start(out=pbk, in_=pbk_d[:].rearrange("(m p) s -> p m s", p=P))
            pbv = cpool.tile([P, KS, D_ATT], FP32, name="pbv_sb")
            nc.sync.dma_start(out=pbv, in_=pbv_d[:].rearrange("(m p) d -> p m d", p=P))
            cf = cpool.tile([P, D_ATT], FP32, name="cf_sb")
            nc.sync.dma_start(out=cf, in_=cf_d[:])
            iota = cpool.tile([P, KS], FP32, name="iota_sb")
            nc.sync.dma_start(out=iota, in_=io_d[:])

            ones_col = cpool.tile([P, 1], BF16, name="ones_col")
            nc.vector.memset(ones_col, 1.0)
            eps_t = cpool.tile([P, 1], FP32, name="eps_t")
            nc.vector.memset(eps_t, EPS)
            zero_t = cpool.tile([P, 1], FP32, name="zero_t")
            nc.vector.memset(zero_t, 0.0)

            # broadcast mask starts to all 128 partitions on GpSimd
            m_row = cpool.tile([1, n_items], FP32, name="m_row")
            nc.sync.dma_start(out=m_row, in_=m_d[:])
            m_bc = cpool.tile([P, n_items], FP32, name="m_bc")
            nc.gpsimd.partition_broadcast(m_bc, m_row)

            # ---------------- per-item pipeline ----------------
            # Software-pipelined emission: each engine's (in-order) stream
            # alternates FRONT(b) and TAIL(b-1) so there is always work from
            # two items in flight and the PE never idles past the HAM window.

            def front(b):
                if b == 0:
                    x_nat = x0
                else:
                    x_nat = wpool.tile(
                        [P, KS, D_IN], FP32, tag="xnat", name=f"xnat{b}"
                    )
                    nc.sync.dma_start(
                        out=x_nat, in_=x_d[b].rearrange("(t p) i -> p t i", p=P)
                    )
                x_bf = wpool.tile([P, KS, D_IN], BF16, tag="xbf", name=f"xbf{b}")
                nc.scalar.copy(x_bf, x_nat)
                # additive key-mask column: 0 where k < m_b else -30000
                maskb = spool.tile([P, KS], FP32, tag="maskb", name=f"maskb{b}")
                nc.vector.tensor_scalar(
                    maskb, iota, m_bc[:, b : b + 1], NEG, OP.is_ge, OP.mult
                )

                # x^T via bf16 PE transposes
                xT = wpool.tile([P, KI, S], BF16, tag="xT", name=f"xT{b}")
                for ii in range(KI):
                    tp = psA.tile([P, S], BF16, tag="ps", name=f"tpx{b}_{ii}")
                    for ss in range(KS):
                        nc.tensor.transpose(
                            out=tp[:, P * ss : P * (ss + 1)],
                            in_=x_bf[:, ss, P * ii : P * (ii + 1)],
                            identity=ident,
                        )
                    nc.scalar.copy(out=xT[:, ii, :], in_=tp)

                # projections: Q^T, K^T as [d, s] bf16; V natural [s, d] fp32
                QT = wpool.tile([P, KD, S], BF16, tag="QT", name=f"QT{b}")
                KTt = wpool.tile([P, KD, S], BF16, tag="KTt", name=f"KTt{b}")
                for dst, w, pb in ((QT, wq, pbq), (KTt, wk, pbk)):
                    for m in range(KD):
                        ps = psA.tile([P, S], FP32, tag="ps")
                        for k in range(KI):
                            nc.tensor.matmul(
                                ps,
                                lhsT=w[:, k, P * m : P * (m + 1)],
                                rhs=xT[:, k, :],
                                start=(k == 0),
                                stop=(k == KI - 1),
                            )
                        nc.vector.tensor_add(dst[:, m, :], ps, pb[:, m, :])
                Vbf = wpool.tile([P, KS, D_ATT], BF16, tag="Vbf", name=f"Vbf{b}")
                for m in range(KS):
                    ps = psA.tile([P, D_ATT], FP32, tag="ps")
                    for k in range(KI):
                        nc.tensor.matmul(
                            ps,
                            lhsT=xT[:, k, P * m : P * (m + 1)],
                            rhs=wv[:, k, :],
                            start=(k == 0),
                            stop=(k == KI - 1),
                        )
                    nc.vector.tensor_add(Vbf[:, m, :], ps, pbv[:, m, :])

                # scores^T [k, q] + fused mask/scale/exp -> E^T (bf16)
                ET = wpool.tile([P, KS, S], BF16, tag="ET", name=f"ET{b}")
                for t in range(KS):
                    ps = psA.tile([P, S], FP32, tag="ps")
                    for m in range(KD):
                        nc.tensor.matmul(
                            ps,
                            lhsT=KTt[:, m, P * t : P * (t + 1)],
                            rhs=QT[:, m, :],
                            start=(m == 0),
                            stop=(m == KD - 1),
                        )
                    nc.scalar.activation(
                        out=ET[:, t, :],
                        in_=ps,
                        func=AF.Exp,
                        bias=maskb[:, t : t + 1],
                        scale=SCALE,
                    )
                return ET, Vbf

            def tail1(b, ET, Vbf):
                # attention output; layernorm of o/den == layernorm of o
                # (row-scale invariance), so no reciprocal: o'' = den*V + num
                # and eps is scaled by den^2 inside the sqrt.
                t2 = wpool.tile([P, KS, D_ATT], BF16, tag="t2", name=f"t2{b}")
                o4 = wpool.tile([P, KS, D_ATT], FP32, tag="o4", name=f"o4{b}")
                den_sb = spool.tile([P, KS], FP32, tag="den_sb", name=f"den{b}")
                ed2 = spool.tile([P, KS], FP32, tag="ed2", name=f"ed2{b}")
                mv4 = spool.tile([P, KS, 2], FP32, tag="mv4", name=f"mv4{b}")
                for m in range(KS):
                    nps = psA.tile([P, D_ATT], FP32, tag="ps")
                    dps = psA.tile([P, 1], FP32, tag="ps")
                    for t in range(KS):
                        lw = ET[:, t, P * m : P * (m + 1)]
                        nc.tensor.matmul(
                            nps, lhsT=lw, rhs=Vbf[:, t, :],
                            start=(t == 0), stop=(t == KS - 1),
                        )
                        nc.tensor.matmul(
                            dps, lhsT=lw, rhs=ones_col,
                            start=(t == 0), stop=(t == KS - 1),
                        )
                    nc.vector.tensor_copy(den_sb[:, m : m + 1], dps)
                    nc.vector.scalar_tensor_tensor(
                        out=o4[:, m, :], in0=Vbf[:, m, :],
                        scalar=den_sb[:, m : m + 1], in1=nps,
                        op0=OP.mult, op1=OP.add,
                    )
                    nc.vector.tensor_scalar(
                        ed2[:, m : m + 1], den_sb[:, m : m + 1],
                        den_sb[:, m : m + 1], EPS, OP.mult, OP.mult,
                    )
                    stats = spool.tile([P, 6], FP32, tag="stats")
                    nc.vector.bn_stats(stats, o4[:, m, :])
                    nc.vector.bn_aggr(mv4[:, m, :], stats)
                # batched per-item LN scalars: one ACT hop for sqrt
                arg4 = spool.tile([P, KS], FP32, tag="arg4", name=f"arg4{b}")
                nc.vector.tensor_add(arg4, mv4[:, :, 1], ed2)
                sd4 = spool.tile([P, KS], FP32, tag="sd4", name=f"sd4{b}")
                nc.scalar.activation(sd4, arg4, AF.Sqrt, bias=zero_t)
                rs4 = spool.tile([P, KS], FP32, tag="rs4", name=f"rs4{b}")
                nc.vector.reciprocal(rs4, sd4)
                nmr4 = spool.tile([P, KS], FP32, tag="nmr4", name=f"nmr4{b}")
                nc.vector.tensor_tensor(nmr4, mv4[:, :, 0], rs4, op=OP.mult)
                nc.vector.tensor_scalar(nmr4, nmr4, -1.0, None, OP.mult)
                for m in range(KS):
                    nc.scalar.activation(
                        out=t2[:, m, :],
                        in_=o4[:, m, :],
                        func=AF.Identity,
                        bias=nmr4[:, m : m + 1],
                        scale=rs4[:, m : m + 1],
                    )

                return t2

            def tail2(b, t2):
                # transpose t2 -> t2T [d, s] (bf16 PE transposes)
                t2T = wpool.tile([P, KD, S], BF16, tag="t2T", name=f"t2T{b}")
                for tt in range(KD):
                    tp = psA.tile([P, S], BF16, tag="ps", name=f"tpt{b}_{tt}")
                    for a in range(KS):
                        nc.tensor.transpose(
                            out=tp[:, P * a : P * (a + 1)],
                            in_=t2[:, a, P * tt : P * (tt + 1)],
                            identity=ident,
                        )
                    nc.scalar.copy(out=t2T[:, tt, :], in_=tp)

                # final: out = t2 @ Wg2 + c
                for m in range(KS):
                    fps = psA.tile([P, D_ATT], FP32, tag="ps")
                    for t in range(KD):
                        nc.tensor.matmul(
                            fps,
                            lhsT=t2T[:, t, P * m : P * (m + 1)],
                            rhs=wg2[:, t, :],
                            start=(t == 0),
                            stop=(t == KD - 1),
                        )
                    out_sb = opool.tile([P, D_ATT], FP32, tag="out_sb")
                    nc.vector.tensor_add(out_sb, fps, cf)
                    nc.sync.dma_start(
                        out=out_d[b, P * m : P * (m + 1), :], in_=out_sb
                    )

            held = {}
            held2 = {}
            for b in range(n_items + 2):
                if b < n_items:
                    held[b] = front(b)
                if b >= 1 and b - 1 < n_items:
                    held2[b - 1] = tail1(b - 1, *held.pop(b - 1))
                if b >= 2:
                    tail2(b - 2, held2.pop(b - 2))
    nc.compile()
    return nc


def host_consts(Wq, bq, Wk, bk, Wv, bv, Wf, bf, pos_emb, gamma, beta):
    """One-time host-side weight-layout transforms (input-data independent)."""
    import ml_dtypes

    f32 = np.float32
    bf16 = ml_dtypes.bfloat16
    pos_emb = np.asarray(pos_emb, f32)[:S]
    gamma = np.asarray(gamma, f32)
    beta = np.asarray(beta, f32)
    Wf = np.asarray(Wf, f32)
    wg2 = gamma[:, None] * Wf + np.diag(gamma).astype(f32)
    c_row = beta @ Wf + np.asarray(bf, f32) + beta
    return {
        "wq": np.ascontiguousarray(np.asarray(Wq, f32).astype(bf16)),
        "wk": np.ascontiguousarray(np.asarray(Wk, f32).astype(bf16)),
        "wv": np.ascontiguousarray(np.asarray(Wv, f32).astype(bf16)),
        "wg2": np.ascontiguousarray(wg2.astype(bf16)),
        "petbq": np.ascontiguousarray(pos_emb.T + np.asarray(bq, f32)[:, None]),
        "petbk": np.ascontiguousarray(pos_emb.T + np.asarray(bk, f32)[:, None]),
        "pebv": np.ascontiguousarray(pos_emb + np.asarray(bv, f32)[None, :]),
        "cfull": np.ascontiguousarray(np.broadcast_to(c_row.astype(f32), (P, D_ATT))),
        "iota4": np.ascontiguousarray(
            (np.arange(P, dtype=f32)[:, None] + P * np.arange(KS, dtype=f32)[None, :])
        ),
        "ident": np.eye(P, dtype=f32).astype(bf16),
    }


_prog_cache = {}


def _get_program(n_items):
    if n_items not in _prog_cache:
        _prog_cache[n_items] = build_program(n_items)
    return _prog_cache[n_items]


def kernel(x, mask_start, Wq, bq, Wk, bk, Wv, bv, Wf, bf, pos_emb, gamma, beta):
    global LAST_RESULTS
    x = np.asarray(x, np.float32)
    mask_f = np.asarray(mask_start).astype(np.float32)
    consts = host_consts(Wq, bq, Wk, bk, Wv, bv, Wf, bf, pos_emb, gamma, beta)

    nc = _get_program(BPC)
    in_maps = []
    for c in range(N_CORES):
        m = dict(consts)
        m["x"] = np.ascontiguousarray(x[c * BPC : (c + 1) * BPC])
        m["mstart"] = np.ascontiguousarray(mask_f[c * BPC : (c + 1) * BPC])[None, :]
        in_maps.append(m)

    res = run_bass_kernel_spmd(nc, in_maps, core_ids=list(range(N_CORES)), trace=TRACE)
    LAST_RESULTS = res
    out = np.concatenate([res.results[c]["out"] for c in range(N_CORES)], axis=0)
    return out
